# revision 1
# baseline (speedup 1.0000x reference)
"""CrossWinAttention Trainium2 kernel.

Computes, for each of 256 independent (x,y) windows:
  LN -> Q/K/V projections -> 4-head attention over T=384 tokens
  -> output projection -> mean over the N=6 slices.

Sharding: 8 cores x 32 windows (2 x-rows of the 16x16 window grid per core).
LN affine and all linear biases are folded into the weights host-side
(parameter-only transforms); the 1/N mean is folded into Wp.

Per-window device dataflow (all fp32):
  DMA natural [T,128] tokens -> bn_stats LN stats -> rstd via ln/exp (ACT,
  same table set as softmax exp) -> fused (x-m)*rstd -> PE transpose ->
  x^T -> projections (Q^T, K^T transposed-out; V natural-out) ->
  row-packed score matmuls S^T (4 heads, K=32 row groups) -> batched exp
  from PSUM with 1/sqrt(dh) folded into the activation scale ->
  ones-matmul denominators -> reciprocal via DMA-reshape to [128,12] ->
  broadcast matmul -> normalize fused into the a^T bounce -> col-packed
  AV matmuls -> output projection with the n-mean folded (6 accumulating
  matmuls) -> PE transpose -> DMA out.
"""

import numpy as np

import concourse.bass as bass
import concourse.tile as tile
from concourse import mybir
from concourse.bass_utils import run_bass_kernel_spmd

# Problem shape (hardcoded per spec)
B, N, X, Y, W1, W2 = 1, 6, 16, 16, 8, 8
DIM, HEADS, DH = 128, 4, 32
INNER = HEADS * DH
T = N * W1 * W2          # 384 tokens per window
WQ = W1 * W2             # 64 tokens per n-slice
NCORES = 8
WIN_PER_CORE = (X // NCORES) * Y   # 32
EPS = 1e-5
SCALE = DH ** -0.5
F32 = mybir.dt.float32
ACT = mybir.ActivationFunctionType
ALU = mybir.AluOpType

TCH = T // 128           # 3 token chunks of 128

LAST_RESULT = None       # BassKernelResults of the most recent kernel() call


def host_consts():
    ident = np.eye(128, dtype=np.float32)
    mask97 = np.zeros((97, INNER), np.float32)
    for h in range(HEADS):
        mask97[32 * h, 32 * h:32 * h + 32] = 1.0
    ones_col = np.ones((128, 1), np.float32)
    return ident, mask97, ones_col


def build(n_win: int, with_bias: bool, repeat: int = 1):
    """Build the per-core Bass program. Inputs are per-core shards.

    xq/xk/xv: [N, n_win, WQ, DIM]  (token blocks per n-slice per window)
    out:      [n_win, WQ, DIM]
    """
    nc = bass.Bass()

    xq_d = nc.dram_tensor("xq", [n_win, 128, TCH, DIM], F32, kind="ExternalInput")
    xk_d = nc.dram_tensor("xk", [n_win, 128, TCH, DIM], F32, kind="ExternalInput")
    xv_d = nc.dram_tensor("xv", [n_win, 128, TCH, DIM], F32, kind="ExternalInput")
    wq_d = nc.dram_tensor("wq", [DIM, INNER], F32, kind="ExternalInput")
    wk_d = nc.dram_tensor("wk", [DIM, INNER], F32, kind="ExternalInput")
    wv_d = nc.dram_tensor("wv", [DIM, INNER], F32, kind="ExternalInput")
    wp_d = nc.dram_tensor("wp", [INNER, DIM], F32, kind="ExternalInput")
    ident_d = nc.dram_tensor("ident", [128, 128], F32, kind="ExternalInput")
    mask4_d = nc.dram_tensor("mask97", [97, INNER], F32, kind="ExternalInput")
    ones_d = nc.dram_tensor("ones_col", [128, 1], F32, kind="ExternalInput")
    if with_bias:
        bq_d = nc.dram_tensor("bq", [INNER, 1], F32, kind="ExternalInput")
        bk_d = nc.dram_tensor("bk", [INNER, 1], F32, kind="ExternalInput")
        bv_d = nc.dram_tensor("bv_row", [1, INNER], F32, kind="ExternalInput")
        bp_d = nc.dram_tensor("bp6", [DIM, 1], F32, kind="ExternalInput")
    out_d = nc.dram_tensor("out", [n_win, WQ, DIM], F32, kind="ExternalOutput")

    with tile.TileContext(nc) as tc:
        with (
            tc.tile_pool(name="singles", bufs=1) as singles,
            tc.tile_pool(name="xbuf", bufs=6) as xbuf,
            tc.tile_pool(name="stats", bufs=2) as statp,
            tc.tile_pool(name="xt", bufs=2) as xtp,
            tc.tile_pool(name="qkv", bufs=2) as qkvp,
            tc.tile_pool(name="esb", bufs=2) as esbp,
            tc.tile_pool(name="small", bufs=2) as smallp,
            tc.tile_pool(name="zout", bufs=2) as zoutp,
            tc.tile_pool(name="ps_s", bufs=2, space="PSUM") as ps_s,
            tc.tile_pool(name="ps_scr", bufs=2, space="PSUM") as ps_scr,
            tc.tile_pool(name="ps_hold", bufs=1, space="PSUM") as ps_hold,
        ):
            # ---- constants / weights ----
            ident = singles.tile([128, 128], F32)
            nc.sync.dma_start(ident, ident_d[:, :])
            mask97 = singles.tile([97, INNER], F32)
            nc.sync.dma_start(mask97, mask4_d[:, :])
            tiny_sb = singles.tile([97, 1], F32)
            nc.vector.memset(tiny_sb, 1e-30)
            zero97 = singles.tile([97, 1], F32)
            nc.vector.memset(zero97, 0.0)
            ones_col = singles.tile([128, 1], F32)
            nc.sync.dma_start(ones_col, ones_d[:, :])
            eps_sb = singles.tile([128, 1], F32)
            nc.vector.memset(eps_sb, EPS)
            zero_sb = singles.tile([128, 1], F32)
            nc.vector.memset(zero_sb, 0.0)
            zrow_m = singles.tile([1, 128], F32)
            nc.vector.memset(zrow_m, 0.0)
            zrow_n = singles.tile([1, T], F32)
            nc.vector.memset(zrow_n, 0.0)
            w_sb = {}
            for nm, d in (("q", wq_d), ("k", wk_d), ("v", wv_d), ("p", wp_d)):
                w_sb[nm] = singles.tile([128, 128], F32, name=f"w{nm}", tag=f"w{nm}")
                nc.sync.dma_start(w_sb[nm], d[:, :])
            if with_bias:
                bq_sb = singles.tile([INNER, 1], F32)
                nc.sync.dma_start(bq_sb, bq_d[:, :])
                bk_sb = singles.tile([INNER, 1], F32)
                nc.sync.dma_start(bk_sb, bk_d[:, :])
                bv_sb = singles.tile([1, INNER], F32)
                nc.sync.dma_start(bv_sb, bv_d[:, :])
                bp_sb = singles.tile([DIM, 1], F32)
                nc.sync.dma_start(bp_sb, bp_d[:, :])
                ones_row = singles.tile([1, 128], F32)
                nc.vector.memset(ones_row, 1.0)

            for _rep in range(repeat):
              for w in range(n_win):
                # ---- load + LN stats + apply, per tensor ----
                xts = {}   # transposed LN'd inputs in SBUF [128(d), T]
                stats6 = statp.tile([128, 3 * TCH, 6], F32, tag="st6")
                rstd = statp.tile([128, 3 * TCH], F32, tag="rstd")
                x_sb = {}
                for ti, (nm, xd) in enumerate((("q", xq_d), ("k", xk_d), ("v", xv_d))):
                    xt_ = xbuf.tile([128, TCH, DIM], F32, tag="x")
                    x_sb[nm] = xt_
                    nc.sync.dma_start(xt_[:, :, :], xd[w, :, :, :])
                    for c in range(TCH):
                        # self-copy absorbs the DMA-completion wait on DVE
                        # (RAW dep): BN/TR opcodes encode at most one sem wait
                        nc.vector.tensor_copy(xt_[:, c, 0:1], xt_[:, c, 0:1])
                        nc.vector.bn_stats(
                            out=stats6[:, 3 * ti + c, :], in_=xt_[:, c, :]
                        )
                # rstd = exp(-0.5 * ln(var + EPS)); var = stats M2 slot aggregated
                # bn_stats 6-slot layout per group; use bn_aggr for mean/var.
                mv = statp.tile([128, 3 * TCH, 2], F32, tag="mv")
                for g in range(3 * TCH):
                    nc.vector.bn_aggr(out=mv[:, g, :], in_=stats6[:, g, :])
                lnv = statp.tile([128, 3 * TCH], F32, tag="lnv")
                nc.scalar.activation(
                    out=lnv, in_=mv[:, :, 1], func=ACT.Ln, bias=eps_sb, scale=1.0
                )
                nc.scalar.activation(
                    out=rstd, in_=lnv, func=ACT.Exp, bias=zero_sb, scale=-0.5
                )
                for ti, nm in enumerate(("q", "k", "v")):
                    xt_ = x_sb[nm]
                    for c in range(TCH):
                        g = 3 * ti + c
                        nc.vector.tensor_scalar(
                            out=xt_[:, c, :], in0=xt_[:, c, :],
                            scalar1=mv[:, g, 0:1], scalar2=rstd[:, g:g + 1],
                            op0=ALU.subtract, op1=ALU.mult,
                        )
                    # transpose LN'd chunks -> x^T [128(d), T]
                    xtps = ps_scr.tile([128, T], F32, tag="m1")
                    for c in range(TCH):
                        nc.tensor.transpose(
                            xtps[:, 128 * c:128 * (c + 1)], xt_[:, c, :], ident
                        )
                    xts[nm] = xtp.tile([128, T], F32, name=f"xt{nm}", tag=f"xt{nm}")
                    nc.vector.tensor_copy(xts[nm], xtps)

                # ---- projections ----
                qT = qkvp.tile([128, T], F32, tag="qT")
                kT = qkvp.tile([128, T], F32, tag="kT")
                for nm, dst, bias in (("q", qT, "bq"), ("k", kT, "bk")):
                    pps = ps_scr.tile([128, T], F32, tag="m1")
                    nc.tensor.matmul(pps, w_sb[nm], xts[nm])
                    if with_bias:
                        bsb = bq_sb if nm == "q" else bk_sb
                        nc.vector.tensor_scalar(
                            out=dst, in0=pps, scalar1=bsb, scalar2=None, op0=ALU.add
                        )
                    else:
                        nc.vector.tensor_copy(dst, pps)
                v_sb = qkvp.tile([128, TCH, DIM], F32, tag="v")
                vps = ps_scr.tile([128, TCH, DIM], F32, tag="m1")
                for c in range(TCH):
                    nc.tensor.matmul(
                        vps[:, c, :], xts["v"][:, 128 * c:128 * (c + 1)], w_sb["v"]
                    )
                    if with_bias:
                        nc.tensor.matmul(
                            vps[:, c, :], ones_row, bv_sb, start=False
                        )
                nc.vector.tensor_copy(v_sb, vps)

                # ---- attention ----
                aT_ps = ps_hold.tile([128, T], F32, tag="aT")
                den_ps = ps_hold.tile([97, T], F32, tag="den")
                # zero-matmuls: clear has_written + zero the banks so the
                # packed per-head accumulations below can all run start=False
                nc.tensor.matmul(aT_ps, zrow_m, zrow_n, start=True, stop=False,
                                 skip_group_check=True)
                nc.tensor.matmul(den_ps, zrow_m[:, 0:97], zrow_n, start=True,
                                 stop=False, skip_group_check=True)
                for c in range(TCH):
                    e_sb = esbp.tile([128, HEADS, T], F32, tag="E")
                    for g in range(2):  # head pairs
                        s_ps = ps_s.tile([128, 2, 512], F32, tag="S")
                        for hh in range(2):
                            h = 2 * g + hh
                            nc.tensor.matmul(
                                s_ps[:, hh, 0:T],
                                kT[32 * h:32 * h + 32, 128 * c:128 * (c + 1)],
                                qT[32 * h:32 * h + 32, :],
                                tile_position=(32 * h, 0),
                            )
                        nc.scalar.activation(
                            out=e_sb[:, 2 * g:2 * g + 2, :], in_=s_ps[:, :, 0:T],
                            func=ACT.Exp, bias=zero_sb, scale=SCALE,
                        )
                    for h in range(HEADS):
                        last = (c == TCH - 1 and h == HEADS - 1)
                        nc.tensor.matmul(
                            den_ps[32 * h:32 * h + 1, :], ones_col, e_sb[:, h, :],
                            start=False, stop=last,
                            tile_position=(0, 32 * h), skip_group_check=True,
                        )
                        nc.tensor.matmul(
                            aT_ps[32 * h:32 * h + 32, :],
                            v_sb[:, c, 32 * h:32 * h + 32], e_sb[:, h, :],
                            start=False, stop=last,
                            tile_position=(0, 32 * h), skip_group_check=True,
                        )

                # ---- softmax denominators -> reciprocal via ACT ln/exp ----
                # recip = exp(-ln(d)); junk rows hold ln(1e-30), masked out by
                # the zero rows of mask97 in the broadcast matmul below
                lden = smallp.tile([97, T], F32, tag="lden")
                nc.scalar.activation(out=lden, in_=den_ps, func=ACT.Ln,
                                     bias=tiny_sb, scale=1.0)
                r97 = smallp.tile([97, T], F32, tag="r97")
                nc.scalar.activation(out=r97, in_=lden, func=ACT.Exp,
                                     bias=zero97, scale=-1.0)
                b_ps = ps_scr.tile([128, T], F32, tag="m1")
                nc.tensor.matmul(b_ps, mask97, r97)
                b_sb = smallp.tile([128, T], F32, tag="bsb")
                nc.vector.tensor_copy(b_sb, b_ps)
                aTn = smallp.tile([128, T], F32, tag="aTn")
                nc.vector.tensor_mul(aTn, aT_ps, b_sb)

                # ---- output projection with folded n-mean, transpose, store ----
                z_ps = ps_scr.tile([128, WQ], F32, tag="m1")
                for n in range(N):
                    nc.tensor.matmul(
                        z_ps, w_sb["p"], aTn[:, WQ * n:WQ * (n + 1)],
                        start=(n == 0), stop=(n == N - 1),
                    )
                zT_sb = zoutp.tile([128, WQ], F32, tag="zT")
                if with_bias:
                    nc.vector.tensor_scalar(
                        out=zT_sb, in0=z_ps, scalar1=bp_sb, scalar2=None, op0=ALU.add
                    )
                else:
                    nc.vector.tensor_copy(zT_sb, z_ps)
                zt_ps = ps_scr.tile([64, 128], F32, tag="m1")
                nc.tensor.transpose(zt_ps, zT_sb, ident)
                zfin = zoutp.tile([64, 128], F32, tag="zfin")
                nc.vector.tensor_copy(zfin, zt_ps)
                nc.gpsimd.dma_start(out_d[w, :, :], zfin)

    return nc


def split_multi_waits(nc):
    """Walrus encodes at most one sem-wait per instruction on this toolchain;
    move extra waits onto same-engine NoOp carriers placed just before."""
    k = 0
    for f in nc.m.functions:
        for blk in f.blocks:
            new = []
            for inst in blk.instructions:
                si = getattr(inst, "sync_info", None)
                if si and si.on_wait and len(si.on_wait) > 1:
                    waits = list(si.on_wait)
                    for w in waits[:-1]:
                        nop = mybir.InstNoOp(
                            name=f"{inst.name}_wsplit{k}", ins=[], outs=[]
                        )
                        k += 1
                        nop.engine = inst.engine
                        nop.sync_info = mybir.SyncInfo(on_wait=[w], on_update=[])
                        new.append(nop)
                    si.on_wait = [waits[-1]]
                new.append(inst)
            blk.instructions[:] = new
    return nc


def kernel(**inputs) -> np.ndarray:
    inp = {k: np.ascontiguousarray(np.asarray(v, dtype=np.float32))
           for k, v in inputs.items()}

    # ---- host parameter folds (weights only) ----
    folded = {}
    for nm in ("q", "k", "v"):
        g = inp[f"ln_{nm}_g"]
        bb = inp[f"ln_{nm}_b"]
        W = inp[f"W{nm}"]
        folded[f"W{nm}"] = np.ascontiguousarray(g[:, None] * W)
        folded[f"b{nm}"] = inp[f"b{nm}"] + bb @ W
    folded["Wp6"] = np.ascontiguousarray(inp["Wp"] / N)
    folded["bp6"] = inp["bp"] / N
    with_bias = any(
        np.abs(folded[b]).max() > 0 for b in ("bq", "bk", "bv", "bp6")
    )

    nc = build(WIN_PER_CORE, with_bias)
    nc.finalize()
    split_multi_waits(nc)

    ident, mask4, ones_col = host_consts()
    base = {
        "wq": folded["Wq"], "wk": folded["Wk"], "wv": folded["Wv"],
        "wp": folded["Wp6"],
        "ident": ident, "mask97": mask4, "ones_col": ones_col,
    }
    if with_bias:
        base["bq"] = folded["bq"].reshape(INNER, 1)
        base["bk"] = folded["bk"].reshape(INNER, 1)
        base["bv_row"] = folded["bv"].reshape(1, INNER)
        base["bp6"] = folded["bp6"].reshape(DIM, 1)

    # shard: core c gets x-rows [2c, 2c+2); windows ordered (xl, y)
    xrows = X // NCORES
    in_maps = []
    for c in range(NCORES):
        m = dict(base)
        for key, nm in (("q", "xq"), ("k", "xk"), ("v", "xv")):
            sh = inp[key][0, :, xrows * c:xrows * (c + 1)]  # [N, 2, Y, W1, W2, D]
            sh = sh.reshape(N, WIN_PER_CORE, WQ, DIM).transpose(1, 0, 2, 3)
            sh = sh.reshape(WIN_PER_CORE, TCH, 128, DIM).transpose(0, 2, 1, 3)
            m[nm] = np.ascontiguousarray(sh)
        in_maps.append(m)

    res = run_bass_kernel_spmd(nc, in_maps, core_ids=list(range(NCORES)))
    global LAST_RESULT
    LAST_RESULT = res
    outs = res.results
    full = np.zeros((B, X, Y, W1, W2, DIM), np.float32)
    for c in range(NCORES):
        o = np.asarray(outs[c]["out"]).reshape(xrows, Y, W1, W2, DIM)
        full[0, xrows * c:xrows * (c + 1)] = o
    return full



# revision 10
# speedup vs baseline: 2.6387x; 2.6387x over previous
"""CrossWinAttention Trainium2 kernel (bf16 matmul datapath).

Computes, for each of 256 independent (x,y) windows:
  LN -> Q/K/V projections -> 4-head attention over T=384 tokens
  -> output projection -> mean over the N=6 slices.

Sharding: 8 cores x 32 windows (2 x-rows of the 16x16 window grid per core).
LN affine and all linear biases are folded into the weights host-side
(parameter-only transforms); the 1/N mean is folded into Wp.

Per-window device dataflow:
  DMA natural [T,128] fp32 tokens -> batched bn_stats (one per tensor) ->
  rstd via ACT ln/exp -> LN apply on DVE writing BF16 -> PE transpose
  (bf16 identity, 1 cyc/row) -> x^T bf16 -> projections with bf16
  weights (PSUM fp32) -> copies cast to bf16 (xT/v on GPSIMD, qT/kT on
  DVE) -> score matmuls S^T (4 heads, bf16, 1 cyc/row) -> exp from PSUM
  with 1/sqrt(dh) folded into the ACT scale, output bf16 -> ones-matmul
  denominators packed at rows 32h of a [97,T] PSUM tile (junk rows
  pre-filled with 1.0 by a rank-1 matmul so the plain DVE reciprocal is
  NaN-safe) -> reciprocal on DVE -> broadcast matmul -> normalize fused
  into the a^T bounce -> output projection with the n-mean folded (6
  accumulating bf16 matmuls) -> PE transpose (fp32) -> DMA out.
"""

import ml_dtypes
import numpy as np

import concourse.bass as bass
import concourse.tile as tile
from concourse import mybir
from concourse.bass_utils import run_bass_kernel_spmd

# Problem shape (hardcoded per spec)
B, N, X, Y, W1, W2 = 1, 6, 16, 16, 8, 8
DIM, HEADS, DH = 128, 4, 32
INNER = HEADS * DH
T = N * W1 * W2          # 384 tokens per window
WQ = W1 * W2             # 64 tokens per n-slice
NCORES = 8
WIN_PER_CORE = (X // NCORES) * Y   # 32
EPS = 1e-5
SCALE = DH ** -0.5
F32 = mybir.dt.float32
BF16 = mybir.dt.bfloat16
NPBF = ml_dtypes.bfloat16
ACT = mybir.ActivationFunctionType
ALU = mybir.AluOpType

TCH = T // 128           # 3 token chunks of 128

LAST_RESULT = None       # BassKernelResults of the most recent kernel() call


def host_consts():
    ident32 = np.eye(128, dtype=np.float32)
    identbf = np.eye(128, dtype=np.float32).astype(NPBF)
    mask97 = np.zeros((97, INNER), np.float32)
    for h in range(HEADS):
        mask97[32 * h, 32 * h:32 * h + 32] = 1.0
    mask97 = mask97.astype(NPBF)
    ones_col = np.ones((128, 1), np.float32).astype(NPBF)
    # junk-row indicator: 1.0 on rows that are NOT denominator rows (32h)
    jmask = np.ones((1, 97), np.float32)
    jmask[0, ::32] = 0.0
    jmask = jmask.astype(NPBF)
    ones_row = np.ones((1, T), np.float32).astype(NPBF)
    return ident32, identbf, mask97, ones_col, jmask, ones_row


def build(n_win: int, with_bias: bool, repeat: int = 1):
    """Build the per-core Bass program. Inputs are per-core shards.

    xq/xk/xv: [n_win, 128, TCH, DIM] fp32 (token chunks per window)
    out:      [n_win, WQ, DIM] fp32
    """
    nc = bass.Bass()

    xq_d = nc.dram_tensor("xq", [n_win, 128, TCH, DIM], F32, kind="ExternalInput")
    xk_d = nc.dram_tensor("xk", [n_win, 128, TCH, DIM], F32, kind="ExternalInput")
    xv_d = nc.dram_tensor("xv", [n_win, 128, TCH, DIM], F32, kind="ExternalInput")
    wq_d = nc.dram_tensor("wq", [DIM, INNER], BF16, kind="ExternalInput")
    wk_d = nc.dram_tensor("wk", [DIM, INNER], BF16, kind="ExternalInput")
    wv_d = nc.dram_tensor("wv", [DIM, INNER], BF16, kind="ExternalInput")
    wp_d = nc.dram_tensor("wp", [INNER, DIM], BF16, kind="ExternalInput")
    ident_d = nc.dram_tensor("ident", [128, 128], F32, kind="ExternalInput")
    identbf_d = nc.dram_tensor("identbf", [128, 128], BF16, kind="ExternalInput")
    mask4_d = nc.dram_tensor("mask97", [97, INNER], BF16, kind="ExternalInput")
    ones_d = nc.dram_tensor("ones_col", [128, 1], BF16, kind="ExternalInput")
    jmask_d = nc.dram_tensor("jmask", [1, 97], BF16, kind="ExternalInput")
    onesrow_d = nc.dram_tensor("ones_row", [1, T], BF16, kind="ExternalInput")
    if with_bias:
        bq_d = nc.dram_tensor("bq", [INNER, 1], F32, kind="ExternalInput")
        bk_d = nc.dram_tensor("bk", [INNER, 1], F32, kind="ExternalInput")
        bv_d = nc.dram_tensor("bv_row", [1, INNER], BF16, kind="ExternalInput")
        bp_d = nc.dram_tensor("bp6", [DIM, 1], F32, kind="ExternalInput")
    out_d = nc.dram_tensor("out", [n_win, WQ, DIM], F32, kind="ExternalOutput")

    with tile.TileContext(nc) as tc:
        with (
            tc.tile_pool(name="singles", bufs=1) as singles,
            tc.tile_pool(name="xbuf", bufs=3) as xbuf,
            tc.tile_pool(name="lnb", bufs=2) as lnb,
            tc.tile_pool(name="stats", bufs=2) as statp,
            tc.tile_pool(name="xt", bufs=2) as xtp,
            tc.tile_pool(name="qkv", bufs=2) as qkvp,
            tc.tile_pool(name="esb", bufs=2) as esbp,
            tc.tile_pool(name="small", bufs=2) as smallp,
            tc.tile_pool(name="zout", bufs=2) as zoutp,
            tc.tile_pool(name="ps_s", bufs=2, space="PSUM") as ps_s,
            tc.tile_pool(name="ps_scr", bufs=2, space="PSUM") as ps_scr,
            tc.tile_pool(name="ps_hold", bufs=1, space="PSUM") as ps_hold,
        ):
            # ---- constants / weights ----
            ident = singles.tile([128, 128], F32)
            nc.sync.dma_start(ident, ident_d[:, :])
            identbf = singles.tile([128, 128], BF16)
            nc.sync.dma_start(identbf, identbf_d[:, :])
            mask97 = singles.tile([97, INNER], BF16)
            nc.sync.dma_start(mask97, mask4_d[:, :])
            ones_col = singles.tile([128, 1], BF16)
            nc.sync.dma_start(ones_col, ones_d[:, :])
            jmask = singles.tile([1, 97], BF16)
            nc.sync.dma_start(jmask, jmask_d[:, :])
            ones_row = singles.tile([1, T], BF16)
            nc.sync.dma_start(ones_row, onesrow_d[:, :])
            eps_sb = singles.tile([128, 1], F32)
            nc.vector.memset(eps_sb, EPS)
            zero_sb = singles.tile([128, 1], F32)
            nc.vector.memset(zero_sb, 0.0)
            w_sb = {}
            for nm, d in (("q", wq_d), ("k", wk_d), ("v", wv_d), ("p", wp_d)):
                w_sb[nm] = singles.tile([128, 128], BF16, name=f"w{nm}", tag=f"w{nm}")
                nc.sync.dma_start(w_sb[nm], d[:, :])
            if with_bias:
                bq_sb = singles.tile([INNER, 1], F32)
                nc.sync.dma_start(bq_sb, bq_d[:, :])
                bk_sb = singles.tile([INNER, 1], F32)
                nc.sync.dma_start(bk_sb, bk_d[:, :])
                bv_sb = singles.tile([1, INNER], BF16)
                nc.sync.dma_start(bv_sb, bv_d[:, :])
                bp_sb = singles.tile([DIM, 1], F32)
                nc.sync.dma_start(bp_sb, bp_d[:, :])
                ones_row128 = singles.tile([1, 128], BF16)
                nc.vector.memset(ones_row128, 1.0)

            for _rep in range(repeat):
              for w in range(n_win):
                # ---- load + LN stats (batched) + apply -> bf16 ----
                stats6 = statp.tile([128, 9, 6], F32, tag="st6")
                rstd = statp.tile([128, 9], F32, tag="rstd")
                x_sb = {}
                x_ln = {}
                for ti, (nm, xd) in enumerate((("q", xq_d), ("k", xk_d), ("v", xv_d))):
                    xt_ = xbuf.tile([128, TCH, DIM], F32, tag=f"x{nm}", name=f"x{nm}")
                    x_sb[nm] = xt_
                    nc.sync.dma_start(xt_[:, :, :], xd[w, :, :, :])
                    for c in range(TCH):
                        nc.vector.bn_stats(
                            out=stats6[:, 3 * ti + c, :], in_=xt_[:, c, :]
                        )
                mv = statp.tile([128, 9, 2], F32, tag="mv")
                for g in range(9):
                    nc.vector.bn_aggr(out=mv[:, g, :], in_=stats6[:, g, :])
                # rstd = exp(-0.5 * ln(var + EPS))
                lnv = statp.tile([128, 9], F32, tag="lnv")
                nc.scalar.activation(
                    out=lnv, in_=mv[:, :, 1], func=ACT.Ln, bias=eps_sb, scale=1.0
                )
                nc.scalar.activation(
                    out=rstd, in_=lnv, func=ACT.Exp, bias=zero_sb, scale=-0.5
                )
                for ti, nm in enumerate(("q", "k", "v")):
                    xl = lnb.tile([128, TCH, DIM], BF16, tag=f"xl{nm}", name=f"xl{nm}")
                    x_ln[nm] = xl
                    for c in range(TCH):
                        g = 3 * ti + c
                        nc.vector.tensor_scalar(
                            out=xl[:, c, :], in0=x_sb[nm][:, c, :],
                            scalar1=mv[:, g, 0:1], scalar2=rstd[:, g:g + 1],
                            op0=ALU.subtract, op1=ALU.mult,
                        )

                # ---- transpose LN'd chunks -> x^T bf16 [128(d), T] ----
                xts = {}
                for nm in ("q", "k", "v"):
                    xtps = ps_scr.tile([128, T], BF16, tag="m1b",
                                       padded_shape=[128, 512], bufs=1)
                    with nc.allow_low_precision(
                        reason="pure transpose, no accumulation"
                    ):
                        for c in range(TCH):
                            nc.tensor.transpose(
                                xtps[:, 128 * c:128 * (c + 1)],
                                x_ln[nm][:, c, :], identbf,
                            )
                    xts[nm] = xtp.tile([128, T], BF16, name=f"xt{nm}", tag=f"xt{nm}")
                    nc.vector.tensor_copy(xts[nm], xtps)

                # ---- projections (bf16 weights, fp32 PSUM) ----
                qT = qkvp.tile([128, T], BF16, tag="qT")
                kT = qkvp.tile([128, T], BF16, tag="kT")
                for nm, dst, bsb in (("q", qT, "bq"), ("k", kT, "bk")):
                    pps = ps_scr.tile([128, T], F32, tag="m1")
                    nc.tensor.matmul(pps, w_sb[nm], xts[nm])
                    if with_bias:
                        bb = bq_sb if nm == "q" else bk_sb
                        nc.vector.tensor_scalar(
                            out=dst, in0=pps, scalar1=bb, scalar2=None, op0=ALU.add
                        )
                    else:
                        nc.vector.tensor_copy(dst, pps)
                v_sb = qkvp.tile([128, TCH, DIM], BF16, tag="v")
                vps = ps_scr.tile([128, TCH, DIM], F32, tag="m1")
                for c in range(TCH):
                    nc.tensor.matmul(
                        vps[:, c, :], xts["v"][:, 128 * c:128 * (c + 1)], w_sb["v"]
                    )
                    if with_bias:
                        nc.tensor.matmul(
                            vps[:, c, :], ones_row128, bv_sb, start=False
                        )
                nc.vector.tensor_copy(v_sb, vps)

                # ---- attention ----
                aT_ps = ps_scr.tile([128, T], F32, tag="m1")
                den_ps = ps_hold.tile([97, T], F32, tag="den")
                # rank-1 fill: junk rows (non-32h) of den get 1.0 so the
                # plain reciprocal below stays finite; den rows get 0.
                nc.tensor.matmul(den_ps, jmask, ones_row, start=True, stop=False,
                                 skip_group_check=True)
                for c in range(TCH):
                    e_sb = esbp.tile([128, HEADS, T], BF16, tag="E")
                    for g in range(2):  # head pairs
                        s_ps = ps_s.tile([128, 2, 512], F32, tag="S")
                        for hh in range(2):
                            h = 2 * g + hh
                            nc.tensor.matmul(
                                s_ps[:, hh, 0:T],
                                kT[32 * h:32 * h + 32, 128 * c:128 * (c + 1)],
                                qT[32 * h:32 * h + 32, :],
                                tile_position=(32 * h, 0),
                            )
                        nc.scalar.activation(
                            out=e_sb[:, 2 * g:2 * g + 2, :], in_=s_ps[:, :, 0:T],
                            func=ACT.Exp, bias=zero_sb, scale=SCALE,
                        )
                    for h in range(HEADS):
                        last = (c == TCH - 1 and h == HEADS - 1)
                        nc.tensor.matmul(
                            den_ps[32 * h:32 * h + 1, :], ones_col, e_sb[:, h, :],
                            start=False, stop=last,
                            tile_position=(0, 32 * h), skip_group_check=True,
                        )
                        nc.tensor.matmul(
                            aT_ps[32 * h:32 * h + 32, :],
                            v_sb[:, c, 32 * h:32 * h + 32], e_sb[:, h, :],
                            start=(c == 0), stop=(c == TCH - 1),
                            tile_position=(0, 32 * h), skip_group_check=True,
                        )

                # ---- softmax denominators -> plain DVE reciprocal ----
                r97 = smallp.tile([97, T], BF16, tag="r97")
                with nc.allow_low_precision(reason="softmax recip to bf16"):
                    nc.vector.reciprocal(r97, den_ps)
                b_ps = ps_scr.tile([128, T], F32, tag="m1")
                nc.tensor.matmul(b_ps, mask97, r97)
                b_sb = smallp.tile([128, T], BF16, tag="bsb")
                with nc.allow_low_precision(reason="softmax recip bcast"):
                    nc.vector.tensor_copy(b_sb, b_ps)
                aTn = smallp.tile([128, T], BF16, tag="aTn")
                with nc.allow_low_precision(reason="attn normalize to bf16"):
                    nc.vector.tensor_tensor(
                        out=aTn, in0=aT_ps, in1=b_sb, op=ALU.mult
                    )

                # ---- output projection with folded n-mean, transpose, store ----
                z_ps = ps_scr.tile([128, WQ], F32, tag="m1")
                for n in range(N):
                    nc.tensor.matmul(
                        z_ps, w_sb["p"], aTn[:, WQ * n:WQ * (n + 1)],
                        start=(n == 0), stop=(n == N - 1),
                    )
                zT_sb = zoutp.tile([128, WQ], F32, tag="zT")
                if with_bias:
                    nc.vector.tensor_scalar(
                        out=zT_sb, in0=z_ps, scalar1=bp_sb, scalar2=None, op0=ALU.add
                    )
                else:
                    nc.vector.tensor_copy(zT_sb, z_ps)
                zt_ps = ps_scr.tile([64, 128], F32, tag="m1")
                nc.tensor.transpose(zt_ps, zT_sb, ident)
                zfin = zoutp.tile([64, 128], F32, tag="zfin")
                nc.vector.tensor_copy(zfin, zt_ps)
                nc.gpsimd.dma_start(out_d[w, :, :], zfin)

    return nc


def split_multi_waits(nc):
    """Walrus encodes at most one sem-wait per instruction on this toolchain;
    move extra waits onto same-engine NoOp carriers placed just before."""
    k = 0
    for f in nc.m.functions:
        for blk in f.blocks:
            new = []
            for inst in blk.instructions:
                si = getattr(inst, "sync_info", None)
                if si and si.on_wait and len(si.on_wait) > 1:
                    waits = list(si.on_wait)
                    for w in waits[:-1]:
                        nop = mybir.InstNoOp(
                            name=f"{inst.name}_wsplit{k}", ins=[], outs=[]
                        )
                        k += 1
                        nop.engine = inst.engine
                        nop.sync_info = mybir.SyncInfo(on_wait=[w], on_update=[])
                        new.append(nop)
                    si.on_wait = [waits[-1]]
                new.append(inst)
            blk.instructions[:] = new
    return nc


def kernel(**inputs) -> np.ndarray:
    inp = {k: np.ascontiguousarray(np.asarray(v, dtype=np.float32))
           for k, v in inputs.items()}

    # ---- host parameter folds (weights only) ----
    folded = {}
    for nm in ("q", "k", "v"):
        g = inp[f"ln_{nm}_g"]
        bb = inp[f"ln_{nm}_b"]
        W = inp[f"W{nm}"]
        folded[f"W{nm}"] = np.ascontiguousarray(g[:, None] * W)
        folded[f"b{nm}"] = inp[f"b{nm}"] + bb @ W
    folded["Wp6"] = np.ascontiguousarray(inp["Wp"] / N)
    folded["bp6"] = inp["bp"] / N
    with_bias = any(
        np.abs(folded[b]).max() > 0 for b in ("bq", "bk", "bv", "bp6")
    )

    nc = build(WIN_PER_CORE, with_bias)
    nc.finalize()
    split_multi_waits(nc)

    ident32, identbf, mask97, ones_col, jmask, ones_row = host_consts()
    base = {
        "wq": folded["Wq"].astype(NPBF), "wk": folded["Wk"].astype(NPBF),
        "wv": folded["Wv"].astype(NPBF), "wp": folded["Wp6"].astype(NPBF),
        "ident": ident32, "identbf": identbf, "mask97": mask97,
        "ones_col": ones_col, "jmask": jmask, "ones_row": ones_row,
    }
    if with_bias:
        base["bq"] = folded["bq"].reshape(INNER, 1)
        base["bk"] = folded["bk"].reshape(INNER, 1)
        base["bv_row"] = folded["bv"].reshape(1, INNER).astype(NPBF)
        base["bp6"] = folded["bp6"].reshape(DIM, 1)

    # shard: core c gets x-rows [2c, 2c+2); windows ordered (xl, y)
    xrows = X // NCORES
    in_maps = []
    for c in range(NCORES):
        m = dict(base)
        for key, nm in (("q", "xq"), ("k", "xk"), ("v", "xv")):
            sh = inp[key][0, :, xrows * c:xrows * (c + 1)]  # [N, 2, Y, W1, W2, D]
            sh = sh.reshape(N, WIN_PER_CORE, WQ, DIM).transpose(1, 0, 2, 3)
            sh = sh.reshape(WIN_PER_CORE, TCH, 128, DIM).transpose(0, 2, 1, 3)
            m[nm] = np.ascontiguousarray(sh)
        in_maps.append(m)

    res = run_bass_kernel_spmd(nc, in_maps, core_ids=list(range(NCORES)))
    global LAST_RESULT
    LAST_RESULT = res
    outs = res.results
    full = np.zeros((B, X, Y, W1, W2, DIM), np.float32)
    for c in range(NCORES):
        o = np.asarray(outs[c]["out"]).reshape(xrows, Y, W1, W2, DIM)
        full[0, xrows * c:xrows * (c + 1)] = o
    return full
